# revision 37
# baseline (speedup 1.0000x reference)
"""Trainium2 Bass kernel for the masked-attention module.

Math (per batch row b):
    att_h = h @ W_h2att.T + b_h2att                       # [A]
    dot_l = sum_a tanh(f2[l,a] + att_h[a]) * w_alpha[a]   # [L]  (b_alpha cancels)
    m     = exp(dot) * mask      # softmax denominator cancels with masked renorm
    out   = (sum_l m[l] * f1[l,:]) / sum_l m[l]           # [D]

Sharding: data-parallel over B across 8 NeuronCores (16 rows each); weights
replicated.  Inputs are downcast to bf16 on the host (fp32 matmuls run at
1/4 rate on TRN2 and fp32 doubles the DMA bytes); accumulations are fp32.

att_feats2 is transposed on the host to [B, A, L] so that on-chip tiles put
the attention dim A on partitions: the ScalarEngine then fuses the att_h add
into tanh via its per-partition bias operand, and the TensorEngine contracts
over A (partitions) against w_alpha for the dot — the VectorEngine stays
nearly idle instead of being the bottleneck.  Verified rel err ~3e-3.
"""

import numpy as np

import concourse.bacc as bacc
import concourse.bass as bass
import concourse.mybir as mybir
import concourse.tile as tile
from concourse.bass import ts
from concourse.bass_utils import run_bass_kernel_spmd

# Problem geometry (hardcoded per spec).
B, L, RNN, ATT = 128, 1024, 1024, 512
N_CORES = 8
BS = B // N_CORES          # 16 batch rows per core
P = 128                    # partitions
F32 = mybir.dt.float32
BF16 = mybir.dt.bfloat16
AF = mybir.ActivationFunctionType
ALU = mybir.AluOpType


def build_nc(BS=BS, L=L, RNN=RNN, ATT=ATT):
    LC = L // P            # l-chunks
    RC = RNN // P          # r-chunks
    AC = ATT // P          # a-chunks
    HLC = LC // 2          # l-chunks per f1 half
    nc = bacc.Bacc("TRN2", target_bir_lowering=False, debug=False)

    # hT[r, b] = h[b, r] (host-transposed)
    h_d = nc.dram_tensor("hT", [RNN, BS], BF16, kind="ExternalInput").ap()
    f1_d = nc.dram_tensor("att_feats1", [BS, L, RNN], BF16, kind="ExternalInput").ap()
    # host-transposed: f2T[b, a, l'] = att_feats2[b, l, a] (l' = permuted l)
    f2_d = nc.dram_tensor("att_feats2T", [BS, ATT, L], BF16, kind="ExternalInput").ap()
    # host-prepared transposed mask: maskT[p, b*LC + c] = mask[b, l(c, p)]
    mask_d = nc.dram_tensor("att_masksT", [P, BS * (L // P)], F32, kind="ExternalInput").ap()
    # WT[r, a] = W[a, r] (host-transposed)
    w_d = nc.dram_tensor("W_h2attT", [RNN, ATT], BF16, kind="ExternalInput").ap()
    bh_d = nc.dram_tensor("b_h2att", [ATT], BF16, kind="ExternalInput").ap()
    wa_d = nc.dram_tensor("w_alpha", [ATT], BF16, kind="ExternalInput").ap()
    out_d = nc.dram_tensor("out", [BS, RNN], F32, kind="ExternalOutput").ap()

    with tile.TileContext(nc) as tc:
        with (
            tc.tile_pool(name="singles", bufs=1) as singles,
            tc.tile_pool(name="wn", bufs=2) as wn_pool,
            tc.tile_pool(name="f2", bufs=4) as f2_pool,
            tc.tile_pool(name="f1", bufs=6) as f1_pool,
            tc.tile_pool(name="work", bufs=3) as work_pool,
            tc.tile_pool(name="small", bufs=3) as small_pool,
            tc.tile_pool(name="outp", bufs=3) as out_pool,
            tc.tile_pool(name="psum_misc", bufs=2, space="PSUM") as psum_misc,
            tc.tile_pool(name="psum_dot", bufs=2, space="PSUM") as psum_dot_pool,
            tc.tile_pool(name="psum_out", bufs=2, space="PSUM") as psum_out_pool,
        ):
            # ---------- constants ----------
            ones_row = singles.tile([1, P], BF16)
            nc.vector.memset(ones_row[:], 1.0)
            ones_col = singles.tile([P, 1], F32)    # partition sums
            nc.vector.memset(ones_col[:], 1.0)

            # w_alpha with A on partitions: waT[p, ac] = wa[ac*128 + p]
            waT = singles.tile([P, AC], BF16)
            nc.sync.dma_start(waT[:], wa_d.rearrange("(ac p) -> p ac", p=P))
            bh_sb = singles.tile([1, ATT], BF16)
            nc.sync.dma_start(bh_sb[:], bh_d[None, :])

            # ---------- prologue ----------
            # W^T direct from host-transposed input: WT[:, rc*ATT + a]
            wt_all = singles.tile([P, RC * ATT], BF16)
            nc.sync.dma_start(
                wt_all[:].rearrange("p (rc a) -> p rc a", rc=RC),
                w_d.rearrange("(rc p) a -> p rc a", p=P),
            )
            # h^T direct: ht_all[:, rc*BS + b] = h[b, rc*128 + p]
            ht_all = singles.tile([P, RC * BS], BF16)
            nc.sync.dma_start(
                ht_all[:].rearrange("p (rc b) -> p rc b", rc=RC),
                h_d.rearrange("(rc p) b -> p rc b", p=P),
            )

            # att_h^T with A on partitions: ahT[p, ac*BS + b] = att_h[b, ac*128+p]
            # (fp32, used as the tanh bias)
            ahT = singles.tile([P, AC * BS], F32)
            for ac in range(AC):
                ah_ps = psum_misc.tile([P, BS], F32, tag="misc")
                for rc in range(RC):
                    nc.tensor.matmul(
                        ah_ps[:],
                        wt_all[:, rc * ATT + ac * P : rc * ATT + (ac + 1) * P],
                        ht_all[:, ts(rc, BS)],
                        start=(rc == 0),
                        stop=False,
                    )
                # + b_h2att: K=1 matmul, lhsT = bh chunk row, rhs = ones
                nc.tensor.matmul(
                    ah_ps[:],
                    bh_sb[:, ts(ac, P)],
                    ones_row[:, :BS],
                    start=False,
                    stop=True,
                )
                nc.vector.tensor_copy(ahT[:, ts(ac, BS)], ah_ps[:])

            # transposed mask direct from host: maskT[p, b*LC + c] = mask[b, l(c,p)]
            maskT = singles.tile([P, BS * LC], F32)
            nc.sync.dma_start(maskT[:], mask_d[:])

            # ---------- per-batch software pipeline ----------
            # Engines are in-order; stages of consecutive batches are emitted
            # interleaved so no engine's next instruction waits on a result
            # another engine only just started producing.
            f2t_h = {}
            tanh_h = {}
            f1t_h = {}
            dotrow_h = {}
            mw_h = {}
            rsum_h = {}

            def emit_load(b):
                # f2T[b] in one 1 MiB DMA: [128, AC, L], a = ac*128 + p
                f2t = f2_pool.tile([P, AC, L], BF16, tag="f2")
                nc.sync.dma_start(
                    f2t[:], f2_d[b].rearrange("(ac p) l -> p ac l", p=P)
                )
                f2t_h[b] = f2t
                # f1[b] in two 1 MiB DMAs: [128, HLC, RNN], l = c*128 + p
                for half in range(2):
                    # contiguous 8 KiB per partition: l = half*512 + 4*p + ci
                    f1t = f1_pool.tile([P, HLC, RNN], BF16, tag="f1")
                    nc.sync.dma_start(
                        f1t[:],
                        f1_d[b, half * HLC * P : (half + 1) * HLC * P].rearrange(
                            "(p ci) d -> p ci d", p=P
                        ),
                    )
                    f1t_h[(b, half)] = f1t

            def emit_tanh(b):
                f2t = f2t_h.pop(b)
                tt = work_pool.tile([P, AC, L], BF16, tag="tanh")
                for ac in range(AC):
                    nc.scalar.activation(
                        tt[:, ac, :],
                        f2t[:, ac, :],
                        AF.Tanh,
                        bias=ahT[:, ac * BS + b : ac * BS + b + 1],
                    )
                tanh_h[b] = tt

            def emit_dot(b):
                tt = tanh_h.pop(b)
                # dot in "swap" form: the tanh tile is the stationary operand
                # (M=128 l-columns), w_alpha the moving one (N=1) — the result
                # lands directly in [l%128, chunk] layout, no transpose-back,
                # and the dense LDWEIGHTS stream keeps the PE HAM-warm.
                dotT_ps = psum_dot_pool.tile([P, LC], F32, tag="dot")
                for c in range(LC):
                    for ac in range(AC):
                        nc.tensor.matmul(
                            dotT_ps[:, c : c + 1],
                            tt[:, ac, ts(c, P)],
                            waT[:, ac : ac + 1],
                            start=(ac == 0),
                            stop=(ac == AC - 1),
                        )
                dotrow_h[b] = dotT_ps

            def emit_softmax(b):
                dotT_ps = dotrow_h.pop(b)
                e_b = small_pool.tile([P, LC], F32, tag="eb")
                nc.scalar.activation(e_b[:], dotT_ps[:], AF.Exp)
                m_b = small_pool.tile([P, LC], F32, tag="mb")
                nc.vector.tensor_mul(m_b[:], e_b[:], maskT[:, ts(b, LC)])
                mw_b = small_pool.tile([P, LC], BF16, tag="mwb")
                nc.vector.tensor_copy(mw_b[:], m_b[:])
                s_b = small_pool.tile([P, 1], F32, tag="sb")
                nc.vector.tensor_reduce(
                    s_b[:], m_b[:], axis=mybir.AxisListType.X, op=ALU.add
                )
                ssum_ps = psum_misc.tile([1, 1], F32, tag="misc")
                nc.tensor.matmul(ssum_ps[:], ones_col[:], s_b[:], start=True, stop=True)
                rsum = small_pool.tile([1, 1], F32, tag="rsum")
                nc.vector.reciprocal(rsum[:], ssum_ps[:])
                mw_h[b] = mw_b
                rsum_h[b] = rsum

            def emit_out(b):
                mw_b = mw_h.pop(b)
                o_ps = psum_out_pool.tile([1, RNN], F32, tag="out")
                d_chunk = min(512, RNN)
                for half in range(2):
                    f1t = f1t_h.pop((b, half))
                    for ci in range(HLC):
                        c = half * HLC + ci
                        w_col = mw_b[:, c : c + 1]
                        for dc in range(RNN // d_chunk):
                            nc.tensor.matmul(
                                o_ps[:, ts(dc, d_chunk)],
                                w_col,
                                f1t[:, ci, ts(dc, d_chunk)],
                                start=(c == 0),
                                stop=(c == LC - 1),
                            )
                # normalize during the PSUM->SBUF copy: out = in * (1/sum)
                o_sb = out_pool.tile([1, RNN], F32, tag="osb")
                nc.vector.tensor_scalar_mul(o_sb[:], o_ps[:], rsum_h.pop(b)[:])
                nc.scalar.dma_start(out_d[b][None, :], o_sb[:])

            for it in range(BS + 4):
                if it < BS:
                    emit_load(it)
                if 1 <= it and it - 1 < BS:
                    emit_tanh(it - 1)
                if 2 <= it and it - 2 < BS:
                    emit_dot(it - 2)
                if 3 <= it and it - 3 < BS:
                    emit_softmax(it - 3)
                if 4 <= it and it - 4 < BS:
                    emit_out(it - 4)

    nc.compile()
    return nc


_NC_CACHE = None


def _get_nc():
    global _NC_CACHE
    if _NC_CACHE is None:
        _NC_CACHE = build_nc()
    return _NC_CACHE


def _prep_f2T(f2, L=L, P=P):
    """[B, L, A] -> [B, A, L'] where the l axis is permuted to
    l' = c*P + p  <->  l = half*HLC*P + p*HLC + ci  (c = half*HLC + ci)
    matching the contiguous-per-partition f1 tile layout on chip."""
    import ml_dtypes

    Bd, Ld, Ad = f2.shape
    HLC = Ld // P // 2
    f2T = f2.transpose(0, 2, 1)  # [B, A, L]
    f2T = f2T.reshape(Bd, Ad, 2, P, HLC).transpose(0, 1, 2, 4, 3).reshape(Bd, Ad, Ld)
    return np.ascontiguousarray(f2T).astype(ml_dtypes.bfloat16)


def _prep_maskT(mask, L=L, P=P):
    """[BS, L] -> [P, BS*LC] with maskT[p, b*LC + c] = mask[b, l(c, p)],
    l(c, p) = half*HLC*P + p*HLC + ci for c = half*HLC + ci."""
    BSd, Ld = mask.shape
    LC = Ld // P
    HLC = LC // 2
    # mask[b, l] -> [b, half, p, ci] -> [p, b, half, ci] -> [P, BS*LC]
    m = mask.reshape(BSd, 2, P, HLC).transpose(2, 0, 1, 3).reshape(P, BSd * LC)
    return np.ascontiguousarray(m.astype(np.float32))


def _make_in_maps(inputs):
    import ml_dtypes

    bf = lambda x: np.ascontiguousarray(
        np.asarray(x, dtype=np.float32).astype(ml_dtypes.bfloat16)
    )
    h = np.asarray(inputs["h"], dtype=np.float32)
    hT = bf(h.T)
    f1 = bf(inputs["att_feats1"])
    f2T = _prep_f2T(np.asarray(inputs["att_feats2"], dtype=np.float32))
    mask = np.asarray(inputs["att_masks"], dtype=np.float32)
    wT = bf(np.asarray(inputs["W_h2att"], dtype=np.float32).T)
    bh = bf(inputs["b_h2att"])
    wa = bf(inputs["w_alpha"])
    in_maps = []
    for i in range(N_CORES):
        sl = slice(i * BS, (i + 1) * BS)
        in_maps.append(
            {
                "hT": np.ascontiguousarray(hT[:, sl]),
                "att_feats1": f1[sl],
                "att_feats2T": f2T[sl],
                "att_masksT": _prep_maskT(mask[sl]),
                "W_h2attT": wT,
                "b_h2att": bh,
                "w_alpha": wa,
            }
        )
    return in_maps


def _ensure_ntff_hook():
    """The agent image's antenv lacks axon_hooks; shim it so trace=True can
    capture NTFF profiles through libaxon_pjrt's ctypes interface."""
    import sys
    import types

    try:
        import antenv.axon_hooks  # noqa: F401
        return
    except ImportError:
        pass
    try:
        from trn_agent_boot.trn_boot import _ntff_profile_via_ctypes

        hook = _ntff_profile_via_ctypes("/opt/axon/libaxon_pjrt.so")
    except Exception:
        hook = None
    mod = types.ModuleType("antenv.axon_hooks")
    mod._hook = hook
    mod.get_axon_ntff_profile_hook = lambda: mod._hook
    mod.set_axon_ntff_profile_hook = lambda h: setattr(mod, "_hook", h)
    sys.modules["antenv.axon_hooks"] = mod


def run(inputs, trace=False):
    """Returns (full_output [B, RNN] float32, exec_time_ns or None)."""
    if trace:
        _ensure_ntff_hook()
    nc = _get_nc()
    res = run_bass_kernel_spmd(
        nc, _make_in_maps(inputs), core_ids=list(range(N_CORES)), trace=trace
    )
    out = np.concatenate([r["out"] for r in res.results], axis=0)
    return out.astype(np.float32), res.exec_time_ns


def kernel(**inputs):
    out, _ = run(inputs, trace=False)
    return out


# revision 38
# speedup vs baseline: 1.0508x; 1.0508x over previous
"""Trainium2 Bass kernel for the masked-attention module.

Math (per batch row b):
    att_h = h @ W_h2att.T + b_h2att                       # [A]
    dot_l = sum_a tanh(f2[l,a] + att_h[a]) * w_alpha[a]   # [L]  (b_alpha cancels)
    m     = exp(dot) * mask      # softmax denominator cancels with masked renorm
    out   = (sum_l m[l] * f1[l,:]) / sum_l m[l]           # [D]

Sharding: data-parallel over B across 8 NeuronCores (16 rows each); weights
replicated.  Inputs are downcast to bf16 on the host (fp32 matmuls run at
1/4 rate on TRN2 and fp32 doubles the DMA bytes); accumulations are fp32.

att_feats2 is transposed on the host to [B, A, L] so that on-chip tiles put
the attention dim A on partitions: the ScalarEngine then fuses the att_h add
into tanh via its per-partition bias operand, and the TensorEngine contracts
over A (partitions) against w_alpha for the dot — the VectorEngine stays
nearly idle instead of being the bottleneck.  Verified rel err ~3e-3.
"""

import numpy as np

import concourse.bacc as bacc
import concourse.bass as bass
import concourse.mybir as mybir
import concourse.tile as tile
from concourse.bass import ts
from concourse.bass_utils import run_bass_kernel_spmd

# Problem geometry (hardcoded per spec).
B, L, RNN, ATT = 128, 1024, 1024, 512
N_CORES = 8
BS = B // N_CORES          # 16 batch rows per core
P = 128                    # partitions
F32 = mybir.dt.float32
BF16 = mybir.dt.bfloat16
AF = mybir.ActivationFunctionType
ALU = mybir.AluOpType


def build_nc(BS=BS, L=L, RNN=RNN, ATT=ATT):
    LC = L // P            # l-chunks
    RC = RNN // P          # r-chunks
    AC = ATT // P          # a-chunks
    HLC = LC // 2          # l-chunks per f1 half
    nc = bacc.Bacc("TRN2", target_bir_lowering=False, debug=False)

    # hT[r, b] = h[b, r] (host-transposed)
    h_d = nc.dram_tensor("hT", [RNN, BS], BF16, kind="ExternalInput").ap()
    f1_d = nc.dram_tensor("att_feats1", [BS, L, RNN], BF16, kind="ExternalInput").ap()
    # host-transposed: f2T[b, a, l'] = att_feats2[b, l, a] (l' = permuted l)
    f2_d = nc.dram_tensor("att_feats2T", [BS, ATT, L], BF16, kind="ExternalInput").ap()
    # host-prepared transposed mask: maskT[p, b*LC + c] = mask[b, l(c, p)]
    mask_d = nc.dram_tensor("att_masksT", [P, BS * (L // P)], F32, kind="ExternalInput").ap()
    # WT[r, a] = W[a, r] (host-transposed)
    w_d = nc.dram_tensor("W_h2attT", [RNN, ATT], BF16, kind="ExternalInput").ap()
    bh_d = nc.dram_tensor("b_h2att", [ATT], BF16, kind="ExternalInput").ap()
    wa_d = nc.dram_tensor("w_alpha", [ATT], BF16, kind="ExternalInput").ap()
    out_d = nc.dram_tensor("out", [BS, RNN], F32, kind="ExternalOutput").ap()

    with tile.TileContext(nc) as tc:
        with (
            tc.tile_pool(name="singles", bufs=1) as singles,
            tc.tile_pool(name="wn", bufs=2) as wn_pool,
            tc.tile_pool(name="f2", bufs=4) as f2_pool,
            tc.tile_pool(name="f1", bufs=6) as f1_pool,
            tc.tile_pool(name="work", bufs=3) as work_pool,
            tc.tile_pool(name="small", bufs=3) as small_pool,
            tc.tile_pool(name="outp", bufs=3) as out_pool,
            tc.tile_pool(name="psum_misc", bufs=2, space="PSUM") as psum_misc,
            tc.tile_pool(name="psum_dot", bufs=2, space="PSUM") as psum_dot_pool,
            tc.tile_pool(name="psum_out", bufs=2, space="PSUM") as psum_out_pool,
        ):
            # ---------- constants ----------
            ones_row = singles.tile([1, P], BF16)
            nc.vector.memset(ones_row[:], 1.0)
            ones_col = singles.tile([P, 1], F32)    # partition sums
            nc.vector.memset(ones_col[:], 1.0)

            # w_alpha with A on partitions: waT[p, ac] = wa[ac*128 + p]
            waT = singles.tile([P, AC], BF16)
            nc.sync.dma_start(waT[:], wa_d.rearrange("(ac p) -> p ac", p=P))
            bh_sb = singles.tile([1, ATT], BF16)
            nc.sync.dma_start(bh_sb[:], bh_d[None, :])

            # ---------- prologue ----------
            # W^T direct from host-transposed input: WT[:, rc*ATT + a]
            wt_all = singles.tile([P, RC * ATT], BF16)
            nc.sync.dma_start(
                wt_all[:].rearrange("p (rc a) -> p rc a", rc=RC),
                w_d.rearrange("(rc p) a -> p rc a", p=P),
            )
            # h^T direct: ht_all[:, rc*BS + b] = h[b, rc*128 + p]
            ht_all = singles.tile([P, RC * BS], BF16)
            nc.sync.dma_start(
                ht_all[:].rearrange("p (rc b) -> p rc b", rc=RC),
                h_d.rearrange("(rc p) b -> p rc b", p=P),
            )

            # att_h^T with A on partitions: ahT[p, ac*BS + b] = att_h[b, ac*128+p]
            # (fp32, used as the tanh bias)
            ahT = singles.tile([P, AC * BS], F32)
            for ac in range(AC):
                ah_ps = psum_misc.tile([P, BS], F32, tag="misc")
                for rc in range(RC):
                    nc.tensor.matmul(
                        ah_ps[:],
                        wt_all[:, rc * ATT + ac * P : rc * ATT + (ac + 1) * P],
                        ht_all[:, ts(rc, BS)],
                        start=(rc == 0),
                        stop=False,
                    )
                # + b_h2att: K=1 matmul, lhsT = bh chunk row, rhs = ones
                nc.tensor.matmul(
                    ah_ps[:],
                    bh_sb[:, ts(ac, P)],
                    ones_row[:, :BS],
                    start=False,
                    stop=True,
                )
                nc.vector.tensor_copy(ahT[:, ts(ac, BS)], ah_ps[:])

            # transposed mask direct from host: maskT[p, b*LC + c] = mask[b, l(c,p)]
            maskT = singles.tile([P, BS * LC], F32)
            nc.sync.dma_start(maskT[:], mask_d[:])

            # ---------- per-batch software pipeline ----------
            # Engines are in-order; stages of consecutive batches are emitted
            # interleaved so no engine's next instruction waits on a result
            # another engine only just started producing.
            f2t_h = {}
            tanh_h = {}
            f1t_h = {}
            dotrow_h = {}
            mw_h = {}
            rsum_h = {}

            def emit_load(b):
                # f2T[b] in one 1 MiB DMA: [128, AC, L], a = ac*128 + p
                f2t = f2_pool.tile([P, AC, L], BF16, tag="f2")
                nc.sync.dma_start(
                    f2t[:], f2_d[b].rearrange("(ac p) l -> p ac l", p=P)
                )
                f2t_h[b] = f2t
                # f1[b] in two 1 MiB DMAs: [128, HLC, RNN], l = c*128 + p
                for half in range(2):
                    # contiguous 8 KiB per partition: l = half*512 + 4*p + ci
                    f1t = f1_pool.tile([P, HLC, RNN], BF16, tag="f1")
                    nc.sync.dma_start(
                        f1t[:],
                        f1_d[b, half * HLC * P : (half + 1) * HLC * P].rearrange(
                            "(p ci) d -> p ci d", p=P
                        ),
                    )
                    f1t_h[(b, half)] = f1t

            def emit_tanh(b):
                f2t = f2t_h.pop(b)
                tt = work_pool.tile([P, AC, L], BF16, tag="tanh")
                for ac in range(AC):
                    nc.scalar.activation(
                        tt[:, ac, :],
                        f2t[:, ac, :],
                        AF.Tanh,
                        bias=ahT[:, ac * BS + b : ac * BS + b + 1],
                    )
                tanh_h[b] = tt

            def emit_dot(b):
                tt = tanh_h.pop(b)
                # dot in "swap" form: the tanh tile is the stationary operand
                # (M=128 l-columns), w_alpha the moving one (N=1) — the result
                # lands directly in [l%128, chunk] layout, no transpose-back,
                # and the dense LDWEIGHTS stream keeps the PE HAM-warm.
                dotT_ps = psum_dot_pool.tile([P, LC], F32, tag="dot")
                for c in range(LC):
                    for ac in range(AC):
                        nc.tensor.matmul(
                            dotT_ps[:, c : c + 1],
                            tt[:, ac, ts(c, P)],
                            waT[:, ac : ac + 1],
                            start=(ac == 0),
                            stop=(ac == AC - 1),
                        )
                dotrow_h[b] = dotT_ps

            def emit_softmax(b):
                dotT_ps = dotrow_h.pop(b)
                e_b = small_pool.tile([P, LC], F32, tag="eb")
                nc.scalar.activation(e_b[:], dotT_ps[:], AF.Exp)
                m_b = small_pool.tile([P, LC], F32, tag="mb")
                nc.vector.tensor_mul(m_b[:], e_b[:], maskT[:, ts(b, LC)])
                mw_b = small_pool.tile([P, LC], BF16, tag="mwb")
                nc.vector.tensor_copy(mw_b[:], m_b[:])
                s_b = small_pool.tile([P, 1], F32, tag="sb")
                nc.vector.tensor_reduce(
                    s_b[:], m_b[:], axis=mybir.AxisListType.X, op=ALU.add
                )
                ssum_ps = psum_misc.tile([1, 1], F32, tag="misc")
                nc.tensor.matmul(ssum_ps[:], ones_col[:], s_b[:], start=True, stop=True)
                rsum = small_pool.tile([1, 1], F32, tag="rsum")
                nc.vector.reciprocal(rsum[:], ssum_ps[:])
                mw_h[b] = mw_b
                rsum_h[b] = rsum

            def emit_out(b):
                mw_b = mw_h.pop(b)
                o_ps = psum_out_pool.tile([1, RNN], F32, tag="out")
                d_chunk = min(512, RNN)
                for half in range(2):
                    f1t = f1t_h.pop((b, half))
                    for ci in range(HLC):
                        c = half * HLC + ci
                        w_col = mw_b[:, c : c + 1]
                        for dc in range(RNN // d_chunk):
                            nc.tensor.matmul(
                                o_ps[:, ts(dc, d_chunk)],
                                w_col,
                                f1t[:, ci, ts(dc, d_chunk)],
                                start=(c == 0),
                                stop=(c == LC - 1),
                            )
                # normalize during the PSUM->SBUF copy: out = in * (1/sum)
                o_sb = out_pool.tile([1, RNN], F32, tag="osb")
                nc.vector.tensor_scalar_mul(o_sb[:], o_ps[:], rsum_h.pop(b)[:])
                nc.sync.dma_start(out_d[b][None, :], o_sb[:])

            for it in range(BS + 4):
                if it < BS:
                    emit_load(it)
                if 1 <= it and it - 1 < BS:
                    emit_tanh(it - 1)
                if 2 <= it and it - 2 < BS:
                    emit_dot(it - 2)
                if 3 <= it and it - 3 < BS:
                    emit_softmax(it - 3)
                if 4 <= it and it - 4 < BS:
                    emit_out(it - 4)

    nc.compile()
    return nc


_NC_CACHE = None


def _get_nc():
    global _NC_CACHE
    if _NC_CACHE is None:
        _NC_CACHE = build_nc()
    return _NC_CACHE


def _prep_f2T(f2, L=L, P=P):
    """[B, L, A] -> [B, A, L'] where the l axis is permuted to
    l' = c*P + p  <->  l = half*HLC*P + p*HLC + ci  (c = half*HLC + ci)
    matching the contiguous-per-partition f1 tile layout on chip."""
    import ml_dtypes

    Bd, Ld, Ad = f2.shape
    HLC = Ld // P // 2
    f2T = f2.transpose(0, 2, 1)  # [B, A, L]
    f2T = f2T.reshape(Bd, Ad, 2, P, HLC).transpose(0, 1, 2, 4, 3).reshape(Bd, Ad, Ld)
    return np.ascontiguousarray(f2T).astype(ml_dtypes.bfloat16)


def _prep_maskT(mask, L=L, P=P):
    """[BS, L] -> [P, BS*LC] with maskT[p, b*LC + c] = mask[b, l(c, p)],
    l(c, p) = half*HLC*P + p*HLC + ci for c = half*HLC + ci."""
    BSd, Ld = mask.shape
    LC = Ld // P
    HLC = LC // 2
    # mask[b, l] -> [b, half, p, ci] -> [p, b, half, ci] -> [P, BS*LC]
    m = mask.reshape(BSd, 2, P, HLC).transpose(2, 0, 1, 3).reshape(P, BSd * LC)
    return np.ascontiguousarray(m.astype(np.float32))


def _make_in_maps(inputs):
    import ml_dtypes

    bf = lambda x: np.ascontiguousarray(
        np.asarray(x, dtype=np.float32).astype(ml_dtypes.bfloat16)
    )
    h = np.asarray(inputs["h"], dtype=np.float32)
    hT = bf(h.T)
    f1 = bf(inputs["att_feats1"])
    f2T = _prep_f2T(np.asarray(inputs["att_feats2"], dtype=np.float32))
    mask = np.asarray(inputs["att_masks"], dtype=np.float32)
    wT = bf(np.asarray(inputs["W_h2att"], dtype=np.float32).T)
    bh = bf(inputs["b_h2att"])
    wa = bf(inputs["w_alpha"])
    in_maps = []
    for i in range(N_CORES):
        sl = slice(i * BS, (i + 1) * BS)
        in_maps.append(
            {
                "hT": np.ascontiguousarray(hT[:, sl]),
                "att_feats1": f1[sl],
                "att_feats2T": f2T[sl],
                "att_masksT": _prep_maskT(mask[sl]),
                "W_h2attT": wT,
                "b_h2att": bh,
                "w_alpha": wa,
            }
        )
    return in_maps


def _ensure_ntff_hook():
    """The agent image's antenv lacks axon_hooks; shim it so trace=True can
    capture NTFF profiles through libaxon_pjrt's ctypes interface."""
    import sys
    import types

    try:
        import antenv.axon_hooks  # noqa: F401
        return
    except ImportError:
        pass
    try:
        from trn_agent_boot.trn_boot import _ntff_profile_via_ctypes

        hook = _ntff_profile_via_ctypes("/opt/axon/libaxon_pjrt.so")
    except Exception:
        hook = None
    mod = types.ModuleType("antenv.axon_hooks")
    mod._hook = hook
    mod.get_axon_ntff_profile_hook = lambda: mod._hook
    mod.set_axon_ntff_profile_hook = lambda h: setattr(mod, "_hook", h)
    sys.modules["antenv.axon_hooks"] = mod


def run(inputs, trace=False):
    """Returns (full_output [B, RNN] float32, exec_time_ns or None)."""
    if trace:
        _ensure_ntff_hook()
    nc = _get_nc()
    res = run_bass_kernel_spmd(
        nc, _make_in_maps(inputs), core_ids=list(range(N_CORES)), trace=trace
    )
    out = np.concatenate([r["out"] for r in res.results], axis=0)
    return out.astype(np.float32), res.exec_time_ns


def kernel(**inputs):
    out, _ = run(inputs, trace=False)
    return out


# revision 39
# speedup vs baseline: 1.0637x; 1.0123x over previous
"""Trainium2 Bass kernel for the masked-attention module.

Math (per batch row b):
    att_h = h @ W_h2att.T + b_h2att                       # [A]
    dot_l = sum_a tanh(f2[l,a] + att_h[a]) * w_alpha[a]   # [L]  (b_alpha cancels)
    m     = exp(dot) * mask      # softmax denominator cancels with masked renorm
    out   = (sum_l m[l] * f1[l,:]) / sum_l m[l]           # [D]

Sharding: data-parallel over B across 8 NeuronCores (16 rows each); weights
replicated.  Inputs are downcast to bf16 on the host (fp32 matmuls run at
1/4 rate on TRN2 and fp32 doubles the DMA bytes); accumulations are fp32.

att_feats2 is transposed on the host to [B, A, L] so that on-chip tiles put
the attention dim A on partitions: the ScalarEngine then fuses the att_h add
into tanh via its per-partition bias operand, and the TensorEngine contracts
over A (partitions) against w_alpha for the dot — the VectorEngine stays
nearly idle instead of being the bottleneck.  Verified rel err ~3e-3.
"""

import numpy as np

import concourse.bacc as bacc
import concourse.bass as bass
import concourse.mybir as mybir
import concourse.tile as tile
from concourse.bass import ts
from concourse.bass_utils import run_bass_kernel_spmd

# Problem geometry (hardcoded per spec).
B, L, RNN, ATT = 128, 1024, 1024, 512
N_CORES = 8
BS = B // N_CORES          # 16 batch rows per core
P = 128                    # partitions
F32 = mybir.dt.float32
BF16 = mybir.dt.bfloat16
AF = mybir.ActivationFunctionType
ALU = mybir.AluOpType


def build_nc(BS=BS, L=L, RNN=RNN, ATT=ATT):
    LC = L // P            # l-chunks
    RC = RNN // P          # r-chunks
    AC = ATT // P          # a-chunks
    HLC = LC // 2          # l-chunks per f1 half
    nc = bacc.Bacc("TRN2", target_bir_lowering=False, debug=False)

    # hT[r, b] = h[b, r] (host-transposed)
    h_d = nc.dram_tensor("hT", [RNN, BS], BF16, kind="ExternalInput").ap()
    f1_d = nc.dram_tensor("att_feats1", [BS, L, RNN], BF16, kind="ExternalInput").ap()
    # host-transposed: f2T[b, a, l'] = att_feats2[b, l, a] (l' = permuted l)
    f2_d = nc.dram_tensor("att_feats2T", [BS, ATT, L], BF16, kind="ExternalInput").ap()
    # host-prepared transposed mask: maskT[p, b*LC + c] = mask[b, l(c, p)]
    mask_d = nc.dram_tensor("att_masksT", [P, BS * (L // P)], F32, kind="ExternalInput").ap()
    # WT[r, a] = W[a, r] (host-transposed)
    w_d = nc.dram_tensor("W_h2attT", [RNN, ATT], BF16, kind="ExternalInput").ap()
    bh_d = nc.dram_tensor("b_h2att", [ATT], BF16, kind="ExternalInput").ap()
    wa_d = nc.dram_tensor("w_alpha", [ATT], BF16, kind="ExternalInput").ap()
    out_d = nc.dram_tensor("out", [BS, RNN], F32, kind="ExternalOutput").ap()

    with tile.TileContext(nc) as tc:
        with (
            tc.tile_pool(name="singles", bufs=1) as singles,
            tc.tile_pool(name="wn", bufs=2) as wn_pool,
            tc.tile_pool(name="f2", bufs=4) as f2_pool,
            tc.tile_pool(name="f1", bufs=6) as f1_pool,
            tc.tile_pool(name="work", bufs=3) as work_pool,
            tc.tile_pool(name="small", bufs=3) as small_pool,
            tc.tile_pool(name="outp", bufs=3) as out_pool,
            tc.tile_pool(name="psum_misc", bufs=2, space="PSUM") as psum_misc,
            tc.tile_pool(name="psum_dot", bufs=2, space="PSUM") as psum_dot_pool,
            tc.tile_pool(name="psum_out", bufs=2, space="PSUM") as psum_out_pool,
        ):
            # ---------- constants ----------
            ones_row = singles.tile([1, P], BF16)
            nc.vector.memset(ones_row[:], 1.0)
            ones_col = singles.tile([P, 1], F32)    # partition sums
            nc.vector.memset(ones_col[:], 1.0)

            # w_alpha with A on partitions: waT[p, ac] = wa[ac*128 + p]
            waT = singles.tile([P, AC], BF16)
            nc.sync.dma_start(waT[:], wa_d.rearrange("(ac p) -> p ac", p=P))
            bh_sb = singles.tile([1, ATT], BF16)
            nc.sync.dma_start(bh_sb[:], bh_d[None, :])

            # ---------- prologue ----------
            # W^T direct from host-transposed input: WT[:, rc*ATT + a]
            wt_all = singles.tile([P, RC * ATT], BF16)
            nc.sync.dma_start(
                wt_all[:].rearrange("p (rc a) -> p rc a", rc=RC),
                w_d.rearrange("(rc p) a -> p rc a", p=P),
            )
            # h^T direct: ht_all[:, rc*BS + b] = h[b, rc*128 + p]
            ht_all = singles.tile([P, RC * BS], BF16)
            nc.sync.dma_start(
                ht_all[:].rearrange("p (rc b) -> p rc b", rc=RC),
                h_d.rearrange("(rc p) b -> p rc b", p=P),
            )

            # att_h^T with A on partitions: ahT[p, ac*BS + b] = att_h[b, ac*128+p]
            # (fp32, used as the tanh bias)
            ahT = singles.tile([P, AC * BS], F32)
            for ac in range(AC):
                ah_ps = psum_misc.tile([P, BS], F32, tag="misc")
                for rc in range(RC):
                    nc.tensor.matmul(
                        ah_ps[:],
                        wt_all[:, rc * ATT + ac * P : rc * ATT + (ac + 1) * P],
                        ht_all[:, ts(rc, BS)],
                        start=(rc == 0),
                        stop=False,
                    )
                # + b_h2att: K=1 matmul, lhsT = bh chunk row, rhs = ones
                nc.tensor.matmul(
                    ah_ps[:],
                    bh_sb[:, ts(ac, P)],
                    ones_row[:, :BS],
                    start=False,
                    stop=True,
                )
                nc.vector.tensor_copy(ahT[:, ts(ac, BS)], ah_ps[:])

            # transposed mask direct from host: maskT[p, b*LC + c] = mask[b, l(c,p)]
            maskT = singles.tile([P, BS * LC], F32)
            nc.sync.dma_start(maskT[:], mask_d[:])

            # ---------- per-batch software pipeline ----------
            # Engines are in-order; stages of consecutive batches are emitted
            # interleaved so no engine's next instruction waits on a result
            # another engine only just started producing.
            f2t_h = {}
            tanh_h = {}
            f1t_h = {}
            dotrow_h = {}
            mw_h = {}
            rsum_h = {}

            def emit_load(b):
                # f2T[b] in one 1 MiB DMA: [128, AC, L], a = ac*128 + p
                f2t = f2_pool.tile([P, AC, L], BF16, tag="f2")
                nc.sync.dma_start(
                    f2t[:], f2_d[b].rearrange("(ac p) l -> p ac l", p=P)
                )
                f2t_h[b] = f2t

            def emit_f1load(b):
                # issued one stage later than f2 so the tail batch's softmax
                # chain finishes before its f1 bytes land (FIFO ring order)
                for half in range(2):
                    # contiguous 8 KiB per partition: l = half*512 + 4*p + ci
                    f1t = f1_pool.tile([P, HLC, RNN], BF16, tag="f1")
                    nc.sync.dma_start(
                        f1t[:],
                        f1_d[b, half * HLC * P : (half + 1) * HLC * P].rearrange(
                            "(p ci) d -> p ci d", p=P
                        ),
                    )
                    f1t_h[(b, half)] = f1t

            def emit_tanh(b):
                f2t = f2t_h.pop(b)
                tt = work_pool.tile([P, AC, L], BF16, tag="tanh")
                for ac in range(AC):
                    nc.scalar.activation(
                        tt[:, ac, :],
                        f2t[:, ac, :],
                        AF.Tanh,
                        bias=ahT[:, ac * BS + b : ac * BS + b + 1],
                    )
                tanh_h[b] = tt

            def emit_dot(b):
                tt = tanh_h.pop(b)
                # dot in "swap" form: the tanh tile is the stationary operand
                # (M=128 l-columns), w_alpha the moving one (N=1) — the result
                # lands directly in [l%128, chunk] layout, no transpose-back,
                # and the dense LDWEIGHTS stream keeps the PE HAM-warm.
                dotT_ps = psum_dot_pool.tile([P, LC], F32, tag="dot")
                for c in range(LC):
                    for ac in range(AC):
                        nc.tensor.matmul(
                            dotT_ps[:, c : c + 1],
                            tt[:, ac, ts(c, P)],
                            waT[:, ac : ac + 1],
                            start=(ac == 0),
                            stop=(ac == AC - 1),
                        )
                dotrow_h[b] = dotT_ps

            def emit_softmax(b):
                dotT_ps = dotrow_h.pop(b)
                e_b = small_pool.tile([P, LC], F32, tag="eb")
                nc.scalar.activation(e_b[:], dotT_ps[:], AF.Exp)
                m_b = small_pool.tile([P, LC], F32, tag="mb")
                nc.vector.tensor_mul(m_b[:], e_b[:], maskT[:, ts(b, LC)])
                mw_b = small_pool.tile([P, LC], BF16, tag="mwb")
                nc.vector.tensor_copy(mw_b[:], m_b[:])
                s_b = small_pool.tile([P, 1], F32, tag="sb")
                nc.vector.tensor_reduce(
                    s_b[:], m_b[:], axis=mybir.AxisListType.X, op=ALU.add
                )
                ssum_ps = psum_misc.tile([1, 1], F32, tag="misc")
                nc.tensor.matmul(ssum_ps[:], ones_col[:], s_b[:], start=True, stop=True)
                rsum = small_pool.tile([1, 1], F32, tag="rsum")
                nc.vector.reciprocal(rsum[:], ssum_ps[:])
                mw_h[b] = mw_b
                rsum_h[b] = rsum

            def emit_out(b):
                mw_b = mw_h.pop(b)
                o_ps = psum_out_pool.tile([1, RNN], F32, tag="out")
                d_chunk = min(512, RNN)
                for half in range(2):
                    f1t = f1t_h.pop((b, half))
                    for ci in range(HLC):
                        c = half * HLC + ci
                        w_col = mw_b[:, c : c + 1]
                        for dc in range(RNN // d_chunk):
                            nc.tensor.matmul(
                                o_ps[:, ts(dc, d_chunk)],
                                w_col,
                                f1t[:, ci, ts(dc, d_chunk)],
                                start=(c == 0),
                                stop=(c == LC - 1),
                            )
                # normalize during the PSUM->SBUF copy: out = in * (1/sum)
                o_sb = out_pool.tile([1, RNN], F32, tag="osb")
                nc.vector.tensor_scalar_mul(o_sb[:], o_ps[:], rsum_h.pop(b)[:])
                nc.sync.dma_start(out_d[b][None, :], o_sb[:])

            for it in range(BS + 4):
                if it < BS:
                    emit_load(it)
                if 1 <= it and it - 1 < BS:
                    emit_f1load(it - 1)
                    emit_tanh(it - 1)
                if 2 <= it and it - 2 < BS:
                    emit_dot(it - 2)
                if 3 <= it and it - 3 < BS:
                    emit_softmax(it - 3)
                if 4 <= it and it - 4 < BS:
                    emit_out(it - 4)

    nc.compile()
    return nc


_NC_CACHE = None


def _get_nc():
    global _NC_CACHE
    if _NC_CACHE is None:
        _NC_CACHE = build_nc()
    return _NC_CACHE


def _prep_f2T(f2, L=L, P=P):
    """[B, L, A] -> [B, A, L'] where the l axis is permuted to
    l' = c*P + p  <->  l = half*HLC*P + p*HLC + ci  (c = half*HLC + ci)
    matching the contiguous-per-partition f1 tile layout on chip."""
    import ml_dtypes

    Bd, Ld, Ad = f2.shape
    HLC = Ld // P // 2
    f2T = f2.transpose(0, 2, 1)  # [B, A, L]
    f2T = f2T.reshape(Bd, Ad, 2, P, HLC).transpose(0, 1, 2, 4, 3).reshape(Bd, Ad, Ld)
    return np.ascontiguousarray(f2T).astype(ml_dtypes.bfloat16)


def _prep_maskT(mask, L=L, P=P):
    """[BS, L] -> [P, BS*LC] with maskT[p, b*LC + c] = mask[b, l(c, p)],
    l(c, p) = half*HLC*P + p*HLC + ci for c = half*HLC + ci."""
    BSd, Ld = mask.shape
    LC = Ld // P
    HLC = LC // 2
    # mask[b, l] -> [b, half, p, ci] -> [p, b, half, ci] -> [P, BS*LC]
    m = mask.reshape(BSd, 2, P, HLC).transpose(2, 0, 1, 3).reshape(P, BSd * LC)
    return np.ascontiguousarray(m.astype(np.float32))


def _make_in_maps(inputs):
    import ml_dtypes

    bf = lambda x: np.ascontiguousarray(
        np.asarray(x, dtype=np.float32).astype(ml_dtypes.bfloat16)
    )
    h = np.asarray(inputs["h"], dtype=np.float32)
    hT = bf(h.T)
    f1 = bf(inputs["att_feats1"])
    f2T = _prep_f2T(np.asarray(inputs["att_feats2"], dtype=np.float32))
    mask = np.asarray(inputs["att_masks"], dtype=np.float32)
    wT = bf(np.asarray(inputs["W_h2att"], dtype=np.float32).T)
    bh = bf(inputs["b_h2att"])
    wa = bf(inputs["w_alpha"])
    in_maps = []
    for i in range(N_CORES):
        sl = slice(i * BS, (i + 1) * BS)
        in_maps.append(
            {
                "hT": np.ascontiguousarray(hT[:, sl]),
                "att_feats1": f1[sl],
                "att_feats2T": f2T[sl],
                "att_masksT": _prep_maskT(mask[sl]),
                "W_h2attT": wT,
                "b_h2att": bh,
                "w_alpha": wa,
            }
        )
    return in_maps


def _ensure_ntff_hook():
    """The agent image's antenv lacks axon_hooks; shim it so trace=True can
    capture NTFF profiles through libaxon_pjrt's ctypes interface."""
    import sys
    import types

    try:
        import antenv.axon_hooks  # noqa: F401
        return
    except ImportError:
        pass
    try:
        from trn_agent_boot.trn_boot import _ntff_profile_via_ctypes

        hook = _ntff_profile_via_ctypes("/opt/axon/libaxon_pjrt.so")
    except Exception:
        hook = None
    mod = types.ModuleType("antenv.axon_hooks")
    mod._hook = hook
    mod.get_axon_ntff_profile_hook = lambda: mod._hook
    mod.set_axon_ntff_profile_hook = lambda h: setattr(mod, "_hook", h)
    sys.modules["antenv.axon_hooks"] = mod


def run(inputs, trace=False):
    """Returns (full_output [B, RNN] float32, exec_time_ns or None)."""
    if trace:
        _ensure_ntff_hook()
    nc = _get_nc()
    res = run_bass_kernel_spmd(
        nc, _make_in_maps(inputs), core_ids=list(range(N_CORES)), trace=trace
    )
    out = np.concatenate([r["out"] for r in res.results], axis=0)
    return out.astype(np.float32), res.exec_time_ns


def kernel(**inputs):
    out, _ = run(inputs, trace=False)
    return out


# revision 40
# speedup vs baseline: 1.0723x; 1.0081x over previous
"""Trainium2 Bass kernel for the masked-attention module.

Math (per batch row b):
    att_h = h @ W_h2att.T + b_h2att                       # [A]
    dot_l = sum_a tanh(f2[l,a] + att_h[a]) * w_alpha[a]   # [L]  (b_alpha cancels)
    m     = exp(dot) * mask      # softmax denominator cancels with masked renorm
    out   = (sum_l m[l] * f1[l,:]) / sum_l m[l]           # [D]

Sharding: data-parallel over B across 8 NeuronCores (16 rows each); weights
replicated.  Inputs are downcast to bf16 on the host (fp32 matmuls run at
1/4 rate on TRN2 and fp32 doubles the DMA bytes); accumulations are fp32.

att_feats2 is transposed on the host to [B, A, L] so that on-chip tiles put
the attention dim A on partitions: the ScalarEngine then fuses the att_h add
into tanh via its per-partition bias operand, and the TensorEngine contracts
over A (partitions) against w_alpha for the dot — the VectorEngine stays
nearly idle instead of being the bottleneck.  Verified rel err ~3e-3.
"""

import numpy as np

import concourse.bacc as bacc
import concourse.bass as bass
import concourse.mybir as mybir
import concourse.tile as tile
from concourse.bass import ts
from concourse.bass_utils import run_bass_kernel_spmd

# Problem geometry (hardcoded per spec).
B, L, RNN, ATT = 128, 1024, 1024, 512
N_CORES = 8
BS = B // N_CORES          # 16 batch rows per core
P = 128                    # partitions
F32 = mybir.dt.float32
BF16 = mybir.dt.bfloat16
AF = mybir.ActivationFunctionType
ALU = mybir.AluOpType


def build_nc(BS=BS, L=L, RNN=RNN, ATT=ATT):
    LC = L // P            # l-chunks
    RC = RNN // P          # r-chunks
    AC = ATT // P          # a-chunks
    HLC = LC // 2          # l-chunks per f1 half
    nc = bacc.Bacc("TRN2", target_bir_lowering=False, debug=False)

    # hT[r, b] = h[b, r] (host-transposed)
    h_d = nc.dram_tensor("hT", [RNN, BS], BF16, kind="ExternalInput").ap()
    f1_d = nc.dram_tensor("att_feats1", [BS, L, RNN], BF16, kind="ExternalInput").ap()
    # host-transposed: f2T[b, a, l'] = att_feats2[b, l, a] (l' = permuted l)
    f2_d = nc.dram_tensor("att_feats2T", [BS, ATT, L], BF16, kind="ExternalInput").ap()
    # host-prepared transposed mask: maskT[p, b*LC + c] = mask[b, l(c, p)]
    mask_d = nc.dram_tensor("att_masksT", [P, BS * (L // P)], F32, kind="ExternalInput").ap()
    # WT[r, a] = W[a, r] (host-transposed)
    w_d = nc.dram_tensor("W_h2attT", [RNN, ATT], BF16, kind="ExternalInput").ap()
    bh_d = nc.dram_tensor("b_h2att", [ATT], BF16, kind="ExternalInput").ap()
    wa_d = nc.dram_tensor("w_alpha", [ATT], BF16, kind="ExternalInput").ap()
    out_d = nc.dram_tensor("out", [BS, RNN], F32, kind="ExternalOutput").ap()

    with tile.TileContext(nc) as tc:
        with (
            tc.tile_pool(name="singles", bufs=1) as singles,
            tc.tile_pool(name="wn", bufs=2) as wn_pool,
            tc.tile_pool(name="f2", bufs=4) as f2_pool,
            tc.tile_pool(name="f1", bufs=6) as f1_pool,
            tc.tile_pool(name="work", bufs=3) as work_pool,
            tc.tile_pool(name="small", bufs=3) as small_pool,
            tc.tile_pool(name="outp", bufs=3) as out_pool,
            tc.tile_pool(name="psum_misc", bufs=2, space="PSUM") as psum_misc,
            tc.tile_pool(name="psum_dot", bufs=2, space="PSUM") as psum_dot_pool,
            tc.tile_pool(name="psum_out", bufs=2, space="PSUM") as psum_out_pool,
        ):
            # ---------- constants ----------
            ones_row = singles.tile([1, P], BF16)
            nc.vector.memset(ones_row[:], 1.0)
            ones_col = singles.tile([P, 1], F32)    # partition sums
            nc.vector.memset(ones_col[:], 1.0)

            # w_alpha with A on partitions: waT[p, ac] = wa[ac*128 + p]
            waT = singles.tile([P, AC], BF16)
            nc.sync.dma_start(waT[:], wa_d.rearrange("(ac p) -> p ac", p=P))
            bh_sb = singles.tile([1, ATT], BF16)
            nc.sync.dma_start(bh_sb[:], bh_d[None, :])

            # ---------- prologue ----------
            # W^T direct from host-transposed input: WT[:, rc*ATT + a]
            wt_all = singles.tile([P, RC * ATT], BF16)
            nc.sync.dma_start(
                wt_all[:].rearrange("p (rc a) -> p rc a", rc=RC),
                w_d.rearrange("(rc p) a -> p rc a", p=P),
            )
            # h^T direct: ht_all[:, rc*BS + b] = h[b, rc*128 + p]
            ht_all = singles.tile([P, RC * BS], BF16)
            nc.sync.dma_start(
                ht_all[:].rearrange("p (rc b) -> p rc b", rc=RC),
                h_d.rearrange("(rc p) b -> p rc b", p=P),
            )

            # att_h^T with A on partitions: ahT[p, ac*BS + b] = att_h[b, ac*128+p]
            # (fp32, used as the tanh bias)
            ahT = singles.tile([P, AC * BS], F32)
            for ac in range(AC):
                ah_ps = psum_misc.tile([P, BS], F32, tag="misc")
                for rc in range(RC):
                    nc.tensor.matmul(
                        ah_ps[:],
                        wt_all[:, rc * ATT + ac * P : rc * ATT + (ac + 1) * P],
                        ht_all[:, ts(rc, BS)],
                        start=(rc == 0),
                        stop=False,
                    )
                # + b_h2att: K=1 matmul, lhsT = bh chunk row, rhs = ones
                nc.tensor.matmul(
                    ah_ps[:],
                    bh_sb[:, ts(ac, P)],
                    ones_row[:, :BS],
                    start=False,
                    stop=True,
                )
                nc.vector.tensor_copy(ahT[:, ts(ac, BS)], ah_ps[:])

            # transposed mask direct from host: maskT[p, b*LC + c] = mask[b, l(c,p)]
            maskT = singles.tile([P, BS * LC], F32)
            nc.sync.dma_start(maskT[:], mask_d[:])

            # ---------- per-batch software pipeline ----------
            # Engines are in-order; stages of consecutive batches are emitted
            # interleaved so no engine's next instruction waits on a result
            # another engine only just started producing.
            f2t_h = {}
            tanh_h = {}
            f1t_h = {}
            dotrow_h = {}
            mw_h = {}
            rsum_h = {}

            def emit_load(b):
                # f2T[b] in one 1 MiB DMA: [128, AC, L], a = ac*128 + p
                f2t = f2_pool.tile([P, AC, L], BF16, tag="f2")
                nc.sync.dma_start(
                    f2t[:], f2_d[b].rearrange("(ac p) l -> p ac l", p=P)
                )
                f2t_h[b] = f2t

            def emit_f1load(b):
                # issued one stage later than f2 so the tail batch's softmax
                # chain finishes before its f1 bytes land (FIFO ring order)
                for half in range(2):
                    # contiguous 8 KiB per partition: l = half*512 + 4*p + ci
                    f1t = f1_pool.tile([P, HLC, RNN], BF16, tag="f1")
                    nc.sync.dma_start(
                        f1t[:],
                        f1_d[b, half * HLC * P : (half + 1) * HLC * P].rearrange(
                            "(p ci) d -> p ci d", p=P
                        ),
                    )
                    f1t_h[(b, half)] = f1t

            def emit_tanh(b):
                f2t = f2t_h.pop(b)
                tt = work_pool.tile([P, AC, L], BF16, tag="tanh")
                for ac in range(AC):
                    nc.scalar.activation(
                        tt[:, ac, :],
                        f2t[:, ac, :],
                        AF.Tanh,
                        bias=ahT[:, ac * BS + b : ac * BS + b + 1],
                    )
                tanh_h[b] = tt

            def emit_dot(b):
                tt = tanh_h.pop(b)
                # dot in "swap" form: the tanh tile is the stationary operand
                # (M=128 l-columns), w_alpha the moving one (N=1) — the result
                # lands directly in [l%128, chunk] layout, no transpose-back,
                # and the dense LDWEIGHTS stream keeps the PE HAM-warm.
                dotT_ps = psum_dot_pool.tile([P, LC], F32, tag="dot")
                for c in range(LC):
                    for ac in range(AC):
                        nc.tensor.matmul(
                            dotT_ps[:, c : c + 1],
                            tt[:, ac, ts(c, P)],
                            waT[:, ac : ac + 1],
                            start=(ac == 0),
                            stop=(ac == AC - 1),
                        )
                dotrow_h[b] = dotT_ps

            def emit_softmax(b):
                dotT_ps = dotrow_h.pop(b)
                e_b = small_pool.tile([P, LC], F32, tag="eb")
                nc.scalar.activation(e_b[:], dotT_ps[:], AF.Exp)
                m_b = small_pool.tile([P, LC], F32, tag="mb")
                nc.vector.tensor_mul(m_b[:], e_b[:], maskT[:, ts(b, LC)])
                mw_b = small_pool.tile([P, LC], BF16, tag="mwb")
                nc.vector.tensor_copy(mw_b[:], m_b[:])
                s_b = small_pool.tile([P, 1], F32, tag="sb")
                nc.vector.tensor_reduce(
                    s_b[:], m_b[:], axis=mybir.AxisListType.X, op=ALU.add
                )
                ssum_ps = psum_misc.tile([1, 1], F32, tag="misc")
                nc.tensor.matmul(ssum_ps[:], ones_col[:], s_b[:], start=True, stop=True)
                rsum = small_pool.tile([1, 1], F32, tag="rsum")
                nc.vector.reciprocal(rsum[:], ssum_ps[:])
                mw_h[b] = mw_b
                rsum_h[b] = rsum

            def emit_out(b):
                mw_b = mw_h.pop(b)
                o_ps = psum_out_pool.tile([1, RNN], F32, tag="out")
                d_chunk = min(512, RNN)
                for half in range(2):
                    f1t = f1t_h.pop((b, half))
                    for ci in range(HLC):
                        c = half * HLC + ci
                        w_col = mw_b[:, c : c + 1]
                        for dc in range(RNN // d_chunk):
                            nc.tensor.matmul(
                                o_ps[:, ts(dc, d_chunk)],
                                w_col,
                                f1t[:, ci, ts(dc, d_chunk)],
                                start=(c == 0),
                                stop=(c == LC - 1),
                            )
                # normalize during the PSUM->SBUF copy: out = in * (1/sum)
                o_sb = out_pool.tile([1, RNN], F32, tag="osb")
                nc.vector.tensor_scalar_mul(o_sb[:], o_ps[:], rsum_h.pop(b)[:])
                nc.sync.dma_start(out_d[b][None, :], o_sb[:])

            for it in range(BS + 4):
                if it < BS:
                    emit_load(it)
                if 1 <= it and it - 1 < BS:
                    emit_tanh(it - 1)
                if 2 <= it and it - 2 < BS:
                    emit_f1load(it - 2)
                    emit_dot(it - 2)
                if 3 <= it and it - 3 < BS:
                    emit_softmax(it - 3)
                if 4 <= it and it - 4 < BS:
                    emit_out(it - 4)

    nc.compile()
    return nc


_NC_CACHE = None


def _get_nc():
    global _NC_CACHE
    if _NC_CACHE is None:
        _NC_CACHE = build_nc()
    return _NC_CACHE


def _prep_f2T(f2, L=L, P=P):
    """[B, L, A] -> [B, A, L'] where the l axis is permuted to
    l' = c*P + p  <->  l = half*HLC*P + p*HLC + ci  (c = half*HLC + ci)
    matching the contiguous-per-partition f1 tile layout on chip."""
    import ml_dtypes

    Bd, Ld, Ad = f2.shape
    HLC = Ld // P // 2
    f2T = f2.transpose(0, 2, 1)  # [B, A, L]
    f2T = f2T.reshape(Bd, Ad, 2, P, HLC).transpose(0, 1, 2, 4, 3).reshape(Bd, Ad, Ld)
    return np.ascontiguousarray(f2T).astype(ml_dtypes.bfloat16)


def _prep_maskT(mask, L=L, P=P):
    """[BS, L] -> [P, BS*LC] with maskT[p, b*LC + c] = mask[b, l(c, p)],
    l(c, p) = half*HLC*P + p*HLC + ci for c = half*HLC + ci."""
    BSd, Ld = mask.shape
    LC = Ld // P
    HLC = LC // 2
    # mask[b, l] -> [b, half, p, ci] -> [p, b, half, ci] -> [P, BS*LC]
    m = mask.reshape(BSd, 2, P, HLC).transpose(2, 0, 1, 3).reshape(P, BSd * LC)
    return np.ascontiguousarray(m.astype(np.float32))


def _make_in_maps(inputs):
    import ml_dtypes

    bf = lambda x: np.ascontiguousarray(
        np.asarray(x, dtype=np.float32).astype(ml_dtypes.bfloat16)
    )
    h = np.asarray(inputs["h"], dtype=np.float32)
    hT = bf(h.T)
    f1 = bf(inputs["att_feats1"])
    f2T = _prep_f2T(np.asarray(inputs["att_feats2"], dtype=np.float32))
    mask = np.asarray(inputs["att_masks"], dtype=np.float32)
    wT = bf(np.asarray(inputs["W_h2att"], dtype=np.float32).T)
    bh = bf(inputs["b_h2att"])
    wa = bf(inputs["w_alpha"])
    in_maps = []
    for i in range(N_CORES):
        sl = slice(i * BS, (i + 1) * BS)
        in_maps.append(
            {
                "hT": np.ascontiguousarray(hT[:, sl]),
                "att_feats1": f1[sl],
                "att_feats2T": f2T[sl],
                "att_masksT": _prep_maskT(mask[sl]),
                "W_h2attT": wT,
                "b_h2att": bh,
                "w_alpha": wa,
            }
        )
    return in_maps


def _ensure_ntff_hook():
    """The agent image's antenv lacks axon_hooks; shim it so trace=True can
    capture NTFF profiles through libaxon_pjrt's ctypes interface."""
    import sys
    import types

    try:
        import antenv.axon_hooks  # noqa: F401
        return
    except ImportError:
        pass
    try:
        from trn_agent_boot.trn_boot import _ntff_profile_via_ctypes

        hook = _ntff_profile_via_ctypes("/opt/axon/libaxon_pjrt.so")
    except Exception:
        hook = None
    mod = types.ModuleType("antenv.axon_hooks")
    mod._hook = hook
    mod.get_axon_ntff_profile_hook = lambda: mod._hook
    mod.set_axon_ntff_profile_hook = lambda h: setattr(mod, "_hook", h)
    sys.modules["antenv.axon_hooks"] = mod


def run(inputs, trace=False):
    """Returns (full_output [B, RNN] float32, exec_time_ns or None)."""
    if trace:
        _ensure_ntff_hook()
    nc = _get_nc()
    res = run_bass_kernel_spmd(
        nc, _make_in_maps(inputs), core_ids=list(range(N_CORES)), trace=trace
    )
    out = np.concatenate([r["out"] for r in res.results], axis=0)
    return out.astype(np.float32), res.exec_time_ns


def kernel(**inputs):
    out, _ = run(inputs, trace=False)
    return out


# revision 41
# speedup vs baseline: 1.0979x; 1.0238x over previous
"""Trainium2 Bass kernel for the masked-attention module.

Math (per batch row b):
    att_h = h @ W_h2att.T + b_h2att                       # [A]
    dot_l = sum_a tanh(f2[l,a] + att_h[a]) * w_alpha[a]   # [L]  (b_alpha cancels)
    m     = exp(dot) * mask      # softmax denominator cancels with masked renorm
    out   = (sum_l m[l] * f1[l,:]) / sum_l m[l]           # [D]

Sharding: data-parallel over B across 8 NeuronCores (16 rows each); weights
replicated.  Inputs are downcast to bf16 on the host (fp32 matmuls run at
1/4 rate on TRN2 and fp32 doubles the DMA bytes); accumulations are fp32.

att_feats2 is transposed on the host to [B, A, L] so that on-chip tiles put
the attention dim A on partitions: the ScalarEngine then fuses the att_h add
into tanh via its per-partition bias operand, and the TensorEngine contracts
over A (partitions) against w_alpha for the dot — the VectorEngine stays
nearly idle instead of being the bottleneck.  Verified rel err ~3e-3.
"""

import numpy as np

import concourse.bacc as bacc
import concourse.bass as bass
import concourse.mybir as mybir
import concourse.tile as tile
from concourse.bass import ts
from concourse.bass_utils import run_bass_kernel_spmd

# Problem geometry (hardcoded per spec).
B, L, RNN, ATT = 128, 1024, 1024, 512
N_CORES = 8
BS = B // N_CORES          # 16 batch rows per core
P = 128                    # partitions
F32 = mybir.dt.float32
BF16 = mybir.dt.bfloat16
AF = mybir.ActivationFunctionType
ALU = mybir.AluOpType


def build_nc(BS=BS, L=L, RNN=RNN, ATT=ATT):
    LC = L // P            # l-chunks
    RC = RNN // P          # r-chunks
    AC = ATT // P          # a-chunks
    HLC = LC // 2          # l-chunks per f1 half
    nc = bacc.Bacc("TRN2", target_bir_lowering=False, debug=False)

    # hT[r, b] = h[b, r] (host-transposed)
    h_d = nc.dram_tensor("hT", [RNN, BS], BF16, kind="ExternalInput").ap()
    f1_d = nc.dram_tensor("att_feats1", [BS, L, RNN], BF16, kind="ExternalInput").ap()
    # host-transposed: f2T[b, a, l'] = att_feats2[b, l, a] (l' = permuted l)
    f2_d = nc.dram_tensor("att_feats2T", [BS, ATT, L], BF16, kind="ExternalInput").ap()
    # host-prepared transposed mask: maskT[p, b*LC + c] = mask[b, l(c, p)]
    mask_d = nc.dram_tensor("att_masksT", [P, BS * (L // P)], F32, kind="ExternalInput").ap()
    # WT[r, a] = W[a, r] (host-transposed)
    w_d = nc.dram_tensor("W_h2attT", [RNN, ATT], BF16, kind="ExternalInput").ap()
    bh_d = nc.dram_tensor("b_h2att", [ATT], BF16, kind="ExternalInput").ap()
    wa_d = nc.dram_tensor("w_alpha", [ATT], BF16, kind="ExternalInput").ap()
    out_d = nc.dram_tensor("out", [BS, RNN], F32, kind="ExternalOutput").ap()

    with tile.TileContext(nc) as tc:
        with (
            tc.tile_pool(name="singles", bufs=1) as singles,
            tc.tile_pool(name="wn", bufs=2) as wn_pool,
            tc.tile_pool(name="f2", bufs=4) as f2_pool,
            tc.tile_pool(name="f1", bufs=6) as f1_pool,
            tc.tile_pool(name="work", bufs=3) as work_pool,
            tc.tile_pool(name="small", bufs=3) as small_pool,
            tc.tile_pool(name="outp", bufs=3) as out_pool,
            tc.tile_pool(name="psum_misc", bufs=2, space="PSUM") as psum_misc,
            tc.tile_pool(name="psum_dot", bufs=2, space="PSUM") as psum_dot_pool,
            tc.tile_pool(name="psum_out", bufs=2, space="PSUM") as psum_out_pool,
        ):
            # ---------- constants ----------
            ones_row = singles.tile([1, P], BF16)
            nc.vector.memset(ones_row[:], 1.0)
            ones_col = singles.tile([P, 1], F32)    # partition sums
            nc.vector.memset(ones_col[:], 1.0)

            # w_alpha with A on partitions: waT[p, ac] = wa[ac*128 + p]
            waT = singles.tile([P, AC], BF16)
            nc.sync.dma_start(waT[:], wa_d.rearrange("(ac p) -> p ac", p=P))
            bh_sb = singles.tile([1, ATT], BF16)
            nc.sync.dma_start(bh_sb[:], bh_d[None, :])

            # ---------- prologue ----------
            # W^T direct from host-transposed input: WT[:, rc*ATT + a]
            wt_all = singles.tile([P, RC * ATT], BF16)
            nc.sync.dma_start(
                wt_all[:].rearrange("p (rc a) -> p rc a", rc=RC),
                w_d.rearrange("(rc p) a -> p rc a", p=P),
            )
            # h^T direct: ht_all[:, rc*BS + b] = h[b, rc*128 + p]
            ht_all = singles.tile([P, RC * BS], BF16)
            nc.sync.dma_start(
                ht_all[:].rearrange("p (rc b) -> p rc b", rc=RC),
                h_d.rearrange("(rc p) b -> p rc b", p=P),
            )

            # att_h^T with A on partitions: ahT[p, ac*BS + b] = att_h[b, ac*128+p]
            # (fp32, used as the tanh bias)
            ahT = singles.tile([P, AC * BS], F32)
            for ac in range(AC):
                ah_ps = psum_misc.tile([P, BS], F32, tag="misc")
                for rc in range(RC):
                    nc.tensor.matmul(
                        ah_ps[:],
                        wt_all[:, rc * ATT + ac * P : rc * ATT + (ac + 1) * P],
                        ht_all[:, ts(rc, BS)],
                        start=(rc == 0),
                        stop=False,
                    )
                # + b_h2att: K=1 matmul, lhsT = bh chunk row, rhs = ones
                nc.tensor.matmul(
                    ah_ps[:],
                    bh_sb[:, ts(ac, P)],
                    ones_row[:, :BS],
                    start=False,
                    stop=True,
                )
                nc.vector.tensor_copy(ahT[:, ts(ac, BS)], ah_ps[:])

            # transposed mask direct from host: maskT[p, b*LC + c] = mask[b, l(c,p)]
            maskT = singles.tile([P, BS * LC], F32)
            nc.sync.dma_start(maskT[:], mask_d[:])

            # ---------- per-batch software pipeline ----------
            # Engines are in-order; stages of consecutive batches are emitted
            # interleaved so no engine's next instruction waits on a result
            # another engine only just started producing.
            f2t_h = {}
            tanh_h = {}
            f1t_h = {}
            dotrow_h = {}
            mw_h = {}
            rsum_h = {}

            def emit_load(b):
                # f2T[b] in one 1 MiB DMA: [128, AC, L], a = ac*128 + p
                f2t = f2_pool.tile([P, AC, L], BF16, tag="f2")
                nc.sync.dma_start(
                    f2t[:], f2_d[b].rearrange("(p ac) l -> p ac l", p=P)
                )
                f2t_h[b] = f2t

            def emit_f1load(b):
                # issued one stage later than f2 so the tail batch's softmax
                # chain finishes before its f1 bytes land (FIFO ring order)
                for half in range(2):
                    # contiguous 8 KiB per partition: l = half*512 + 4*p + ci
                    f1t = f1_pool.tile([P, HLC, RNN], BF16, tag="f1")
                    nc.sync.dma_start(
                        f1t[:],
                        f1_d[b, half * HLC * P : (half + 1) * HLC * P].rearrange(
                            "(p ci) d -> p ci d", p=P
                        ),
                    )
                    f1t_h[(b, half)] = f1t

            def emit_tanh(b):
                f2t = f2t_h.pop(b)
                tt = work_pool.tile([P, AC, L], BF16, tag="tanh")
                for ac in range(AC):
                    nc.scalar.activation(
                        tt[:, ac, :],
                        f2t[:, ac, :],
                        AF.Tanh,
                        bias=ahT[:, ac * BS + b : ac * BS + b + 1],
                    )
                tanh_h[b] = tt

            def emit_dot(b):
                tt = tanh_h.pop(b)
                # dot in "swap" form: the tanh tile is the stationary operand
                # (M=128 l-columns), w_alpha the moving one (N=1) — the result
                # lands directly in [l%128, chunk] layout, no transpose-back,
                # and the dense LDWEIGHTS stream keeps the PE HAM-warm.
                dotT_ps = psum_dot_pool.tile([P, LC], F32, tag="dot")
                for c in range(LC):
                    for ac in range(AC):
                        nc.tensor.matmul(
                            dotT_ps[:, c : c + 1],
                            tt[:, ac, ts(c, P)],
                            waT[:, ac : ac + 1],
                            start=(ac == 0),
                            stop=(ac == AC - 1),
                        )
                dotrow_h[b] = dotT_ps

            def emit_softmax(b):
                dotT_ps = dotrow_h.pop(b)
                e_b = small_pool.tile([P, LC], F32, tag="eb")
                nc.scalar.activation(e_b[:], dotT_ps[:], AF.Exp)
                m_b = small_pool.tile([P, LC], F32, tag="mb")
                nc.vector.tensor_mul(m_b[:], e_b[:], maskT[:, ts(b, LC)])
                mw_b = small_pool.tile([P, LC], BF16, tag="mwb")
                nc.vector.tensor_copy(mw_b[:], m_b[:])
                s_b = small_pool.tile([P, 1], F32, tag="sb")
                nc.vector.tensor_reduce(
                    s_b[:], m_b[:], axis=mybir.AxisListType.X, op=ALU.add
                )
                ssum_ps = psum_misc.tile([1, 1], F32, tag="misc")
                nc.tensor.matmul(ssum_ps[:], ones_col[:], s_b[:], start=True, stop=True)
                rsum = small_pool.tile([1, 1], F32, tag="rsum")
                nc.vector.reciprocal(rsum[:], ssum_ps[:])
                mw_h[b] = mw_b
                rsum_h[b] = rsum

            def emit_out(b):
                mw_b = mw_h.pop(b)
                o_ps = psum_out_pool.tile([1, RNN], F32, tag="out")
                d_chunk = min(512, RNN)
                for half in range(2):
                    f1t = f1t_h.pop((b, half))
                    for ci in range(HLC):
                        c = half * HLC + ci
                        w_col = mw_b[:, c : c + 1]
                        for dc in range(RNN // d_chunk):
                            nc.tensor.matmul(
                                o_ps[:, ts(dc, d_chunk)],
                                w_col,
                                f1t[:, ci, ts(dc, d_chunk)],
                                start=(c == 0),
                                stop=(c == LC - 1),
                            )
                # normalize during the PSUM->SBUF copy: out = in * (1/sum)
                o_sb = out_pool.tile([1, RNN], F32, tag="osb")
                nc.vector.tensor_scalar_mul(o_sb[:], o_ps[:], rsum_h.pop(b)[:])
                nc.sync.dma_start(out_d[b][None, :], o_sb[:])

            for it in range(BS + 4):
                if it < BS:
                    emit_load(it)
                if 1 <= it and it - 1 < BS:
                    emit_tanh(it - 1)
                if 2 <= it and it - 2 < BS:
                    emit_f1load(it - 2)
                    emit_dot(it - 2)
                if 3 <= it and it - 3 < BS:
                    emit_softmax(it - 3)
                if 4 <= it and it - 4 < BS:
                    emit_out(it - 4)

    nc.compile()
    return nc


_NC_CACHE = None


def _get_nc():
    global _NC_CACHE
    if _NC_CACHE is None:
        _NC_CACHE = build_nc()
    return _NC_CACHE


def _prep_f2T(f2, L=L, P=P):
    """[B, L, A] -> [B, A, L'] where the l axis is permuted to
    l' = c*P + p  <->  l = half*HLC*P + p*HLC + ci  (c = half*HLC + ci)
    matching the contiguous-per-partition f1 tile layout on chip."""
    import ml_dtypes

    Bd, Ld, Ad = f2.shape
    HLC = Ld // P // 2
    AC = Ad // P
    f2T = f2.transpose(0, 2, 1)  # [B, A, L]
    f2T = f2T.reshape(Bd, Ad, 2, P, HLC).transpose(0, 1, 2, 4, 3).reshape(Bd, Ad, Ld)
    # row order (ac, p) -> (p, ac): one contiguous 8 KiB run per partition
    f2T = f2T.reshape(Bd, AC, P, Ld).transpose(0, 2, 1, 3).reshape(Bd, Ad, Ld)
    return np.ascontiguousarray(f2T).astype(ml_dtypes.bfloat16)


def _prep_maskT(mask, L=L, P=P):
    """[BS, L] -> [P, BS*LC] with maskT[p, b*LC + c] = mask[b, l(c, p)],
    l(c, p) = half*HLC*P + p*HLC + ci for c = half*HLC + ci."""
    BSd, Ld = mask.shape
    LC = Ld // P
    HLC = LC // 2
    # mask[b, l] -> [b, half, p, ci] -> [p, b, half, ci] -> [P, BS*LC]
    m = mask.reshape(BSd, 2, P, HLC).transpose(2, 0, 1, 3).reshape(P, BSd * LC)
    return np.ascontiguousarray(m.astype(np.float32))


def _make_in_maps(inputs):
    import ml_dtypes

    bf = lambda x: np.ascontiguousarray(
        np.asarray(x, dtype=np.float32).astype(ml_dtypes.bfloat16)
    )
    h = np.asarray(inputs["h"], dtype=np.float32)
    hT = bf(h.T)
    f1 = bf(inputs["att_feats1"])
    f2T = _prep_f2T(np.asarray(inputs["att_feats2"], dtype=np.float32))
    mask = np.asarray(inputs["att_masks"], dtype=np.float32)
    wT = bf(np.asarray(inputs["W_h2att"], dtype=np.float32).T)
    bh = bf(inputs["b_h2att"])
    wa = bf(inputs["w_alpha"])
    in_maps = []
    for i in range(N_CORES):
        sl = slice(i * BS, (i + 1) * BS)
        in_maps.append(
            {
                "hT": np.ascontiguousarray(hT[:, sl]),
                "att_feats1": f1[sl],
                "att_feats2T": f2T[sl],
                "att_masksT": _prep_maskT(mask[sl]),
                "W_h2attT": wT,
                "b_h2att": bh,
                "w_alpha": wa,
            }
        )
    return in_maps


def _ensure_ntff_hook():
    """The agent image's antenv lacks axon_hooks; shim it so trace=True can
    capture NTFF profiles through libaxon_pjrt's ctypes interface."""
    import sys
    import types

    try:
        import antenv.axon_hooks  # noqa: F401
        return
    except ImportError:
        pass
    try:
        from trn_agent_boot.trn_boot import _ntff_profile_via_ctypes

        hook = _ntff_profile_via_ctypes("/opt/axon/libaxon_pjrt.so")
    except Exception:
        hook = None
    mod = types.ModuleType("antenv.axon_hooks")
    mod._hook = hook
    mod.get_axon_ntff_profile_hook = lambda: mod._hook
    mod.set_axon_ntff_profile_hook = lambda h: setattr(mod, "_hook", h)
    sys.modules["antenv.axon_hooks"] = mod


def run(inputs, trace=False):
    """Returns (full_output [B, RNN] float32, exec_time_ns or None)."""
    if trace:
        _ensure_ntff_hook()
    nc = _get_nc()
    res = run_bass_kernel_spmd(
        nc, _make_in_maps(inputs), core_ids=list(range(N_CORES)), trace=trace
    )
    out = np.concatenate([r["out"] for r in res.results], axis=0)
    return out.astype(np.float32), res.exec_time_ns


def kernel(**inputs):
    out, _ = run(inputs, trace=False)
    return out
